# revision 1
# baseline (speedup 1.0000x reference)
"""Bass/Trainium2 kernel for batched GNN message passing:
    out[b, d, n] = sum_m adj[b, n, m] * x[b, d, m]
B=2, D=3072, N=8192, fp32.

Sharding: 8 cores, core c -> (b = c//4, n-quarter = c%4). Each core computes
C[d, n_quarter] = X[b] @ A[b, n_quarter, :].T  with D=3072, NC=2048, M=8192.
Zero collectives; host slices inputs and concatenates outputs.

Per-core kernel: K-split into KQ passes. Per K-pass, the full-width adjT panel
[128, KC, NC] is built once via TensorE 128x128 transposes, then a d-block loop
transposes an X strip and runs fp32r matmuls (1 cyc/row) accumulating 4 PSUM
banks over the n-width. Partial outputs accumulate in DRAM across K-passes.
"""

import sys
from contextlib import ExitStack

import numpy as np

sys.path.insert(0, "/opt/trn_rl_repo")

B = 2
D = 3072
N = 8192
NCORES = 8
NSPLIT = 4  # n-quarters per batch sample
NC = N // NSPLIT  # 2048 columns of out per core


def build_program(d=D, ncols=NC, m=N, kq=4, nbw=512):
    """Build the per-core Bass program. Returns compiled nc."""
    import concourse.mybir as mybir
    import concourse.tile as tile
    from concourse import bacc
    from concourse.masks import make_identity

    f32 = mybir.dt.float32
    f32r = mybir.dt.float32r
    P = 128

    mq = m // kq          # contraction elems per K-pass
    kc_n = mq // P        # 128-chunks per K-pass
    n128 = ncols // P     # 128-row blocks of adj slice
    ndb = d // P          # d-blocks
    nnb = ncols // nbw    # psum banks swept per d-block

    nc = bacc.Bacc(None, target_bir_lowering=False, debug=False)

    x_ext = nc.dram_tensor("x", [d, m], f32r, kind="ExternalInput")
    a_ext = nc.dram_tensor("adj", [ncols, m], f32r, kind="ExternalInput")
    out_ext = nc.dram_tensor("out", [d, ncols], f32, kind="ExternalOutput")

    with tile.TileContext(nc) as tc, ExitStack() as ctx:
        dram = ctx.enter_context(tc.tile_pool(name="dram", bufs=1, space="DRAM"))
        c_accum = None
        if kq > 1:
            c_accum = dram.tile([d, ncols], f32, name="c_accum")

        const = ctx.enter_context(tc.tile_pool(name="const", bufs=1))
        ident_f32 = const.tile([P, P], f32)
        make_identity(nc, ident_f32)
        ident = const.tile([P, P], f32r, name="ident_r")
        nc.vector.tensor_copy(out=ident[:], in_=ident_f32[:])

        panel_pool = ctx.enter_context(tc.tile_pool(name="panel", bufs=1))
        xt_pool = ctx.enter_context(tc.tile_pool(name="xt", bufs=2))
        stg_pool = ctx.enter_context(tc.tile_pool(name="stg", bufs=2))
        out_pool = ctx.enter_context(tc.tile_pool(name="outp", bufs=3))
        cprev_pool = ctx.enter_context(tc.tile_pool(name="cprev", bufs=3))
        tp_psum = ctx.enter_context(tc.tile_pool(name="tpp", bufs=2, space="PSUM"))
        acc_psum = ctx.enter_context(tc.tile_pool(name="accp", bufs=6, space="PSUM"))

        for kqi in range(kq):
            mlo = kqi * mq
            # ---- build adjT panel [P, kc_n, ncols] for this K-pass ----
            adjt = panel_pool.tile([P, kc_n, ncols], f32r, tag="adjt")
            for nb in range(n128):
                stg_a = stg_pool.tile([P, mq], f32r, tag="stg_a")
                nc.sync.dma_start(
                    out=stg_a[:], in_=a_ext[nb * P : (nb + 1) * P, mlo : mlo + mq]
                )
                for kc in range(kc_n):
                    pt = tp_psum.tile([P, P], f32r, tag="tp")
                    nc.tensor.transpose(pt[:], stg_a[:, kc * P : (kc + 1) * P], ident[:])
                    nc.vector.tensor_copy(
                        out=adjt[:, kc, nb * P : (nb + 1) * P], in_=pt[:]
                    )

            # ---- d-block loop: transpose X strip, matmul, evict ----
            for db in range(ndb):
                stg_x = stg_pool.tile([P, mq], f32r, tag="stg_x")
                nc.sync.dma_start(
                    out=stg_x[:], in_=x_ext[db * P : (db + 1) * P, mlo : mlo + mq]
                )
                xt = xt_pool.tile([P, kc_n, P], f32r, tag="xt")
                for kc in range(kc_n):
                    pt = tp_psum.tile([P, P], f32r, tag="tp")
                    nc.tensor.transpose(pt[:], stg_x[:, kc * P : (kc + 1) * P], ident[:])
                    nc.vector.tensor_copy(out=xt[:, kc, :], in_=pt[:])

                accs = [
                    acc_psum.tile([P, nbw], f32, tag="acc", name=f"acc{i}")
                    for i in range(nnb)
                ]
                for kc in range(kc_n):
                    for nb in range(nnb):
                        nc.tensor.matmul(
                            accs[nb][:],
                            xt[:, kc, :],
                            adjt[:, kc, nb * nbw : (nb + 1) * nbw],
                            start=(kc == 0),
                            stop=(kc == kc_n - 1),
                        )

                dst = out_ext if kqi == kq - 1 else c_accum
                for nb in range(nnb):
                    osb = out_pool.tile([P, nbw], f32, tag="osb")
                    if kqi == 0:
                        nc.vector.tensor_copy(out=osb[:], in_=accs[nb][:])
                    else:
                        cprev = cprev_pool.tile([P, nbw], f32, tag="cprev")
                        nc.sync.dma_start(
                            out=cprev[:],
                            in_=c_accum[
                                db * P : (db + 1) * P, nb * nbw : (nb + 1) * nbw
                            ],
                        )
                        nc.vector.tensor_tensor(
                            out=osb[:],
                            in0=accs[nb][:],
                            in1=cprev[:],
                            op=mybir.AluOpType.add,
                        )
                    nc.sync.dma_start(
                        out=dst[db * P : (db + 1) * P, nb * nbw : (nb + 1) * nbw],
                        in_=osb[:],
                    )

    nc.compile()
    return nc


_NC_CACHE = {}


def _get_program(**kw):
    key = tuple(sorted(kw.items()))
    if key not in _NC_CACHE:
        _NC_CACHE[key] = build_program(**kw)
    return _NC_CACHE[key]


def kernel(x: np.ndarray, adj: np.ndarray) -> np.ndarray:
    """Full inputs in, full output out. x [B,D,N] f32, adj [B,N,N] f32."""
    from concourse.bass_utils import run_bass_kernel_spmd

    assert x.shape == (B, D, N) and adj.shape == (B, N, N)
    nc = _get_program()

    in_maps = []
    for c in range(NCORES):
        b, ns = divmod(c, NSPLIT)
        in_maps.append(
            {
                "x": np.ascontiguousarray(x[b], dtype=np.float32),
                "adj": np.ascontiguousarray(
                    adj[b, ns * NC : (ns + 1) * NC, :], dtype=np.float32
                ),
            }
        )

    res = run_bass_kernel_spmd(nc, in_maps, core_ids=list(range(NCORES)))
    out = np.empty((B, D, N), dtype=np.float32)
    for c in range(NCORES):
        b, ns = divmod(c, NSPLIT)
        out[b, :, ns * NC : (ns + 1) * NC] = res.results[c]["out"]
    return out



# revision 2
# speedup vs baseline: 1.6476x; 1.6476x over previous
"""Bass/Trainium2 kernel for batched GNN message passing:
    out[b, d, n] = sum_m adj[b, n, m] * x[b, d, m]
B=2, D=3072, N=8192, fp32 in/out.

Sharding: 8 cores, core c -> (b = c//4, n-quarter = c%4). Each core computes
C[3072, 2048] = X[b] @ A[b, quarter, :].T with contraction m = 8192.

Strategy (bf16, zero on-chip transposes, zero DRAM partials):
- Host prepacks both operands transposed + tiled so every DMA is contiguous
  and every matmul operand is already in [contraction-on-partitions] layout.
  bf16 rounding gives rel err ~2e-3 vs the 2e-2 gate (measured on the real
  seeded inputs).
- Per core: 4 n-slabs of 512 cols. Per slab, the full-contraction adj panel
  [128k x 64mc x 512n] (64 KiB/partition) is SBUF-resident (double-buffered
  across slabs -> no PE stall at slab swap). For each of 24 d-blocks, one
  PSUM bank accumulates out[128d, 512n] over all 64 mc chunks in a dense
  back-to-back matmul stream (LDWEIGHTS hides in the PE reorder window, HAM
  stays warm). X d-block strips re-stream per slab (4x50MB, hidden under
  compute).
- Evict: PSUM -> VectorE copy -> SBUF -> DMA out. Panel DMAs issue from the
  ScalarE HWDGE queue so they prefetch ahead of the x/out Sync-queue traffic.
"""

import sys
from contextlib import ExitStack

import numpy as np

sys.path.insert(0, "/opt/trn_rl_repo")

B = 2
D = 3072
N = 8192
NCORES = 8
NSPLIT = 4  # n-quarters per batch sample
NC = N // NSPLIT  # 2048 columns of out per core

P = 128
NDB = D // P  # 24 d-blocks
NMC = N // P  # 64 contraction chunks
NSLAB = 4  # n-slabs per core
NW = NC // NSLAB  # 512 cols per slab


def build_program():
    """Build the per-core Bass program. Returns compiled nc."""
    import concourse.mybir as mybir
    import concourse.tile as tile
    from concourse import bacc

    f32 = mybir.dt.float32
    bf16 = mybir.dt.bfloat16

    nc = bacc.Bacc(None, target_bir_lowering=False, debug=False)

    # xh[db*128 + k, mc*128 + i] = x[b][db*128 + i, mc*128 + k]  (bf16)
    xh = nc.dram_tensor("xh", [D, NMC * P], bf16, kind="ExternalInput")
    # ah[ns*128 + k, mc*512 + j] = adj[b][q*2048 + ns*512 + j, mc*128 + k]
    ah = nc.dram_tensor("ah", [NSLAB * P, NMC * NW], bf16, kind="ExternalInput")
    out_ext = nc.dram_tensor("out", [D, NC], f32, kind="ExternalOutput")

    with tile.TileContext(nc) as tc, ExitStack() as ctx:
        panel_pool = ctx.enter_context(tc.tile_pool(name="panel", bufs=2))
        x_pool = ctx.enter_context(tc.tile_pool(name="xp", bufs=3))
        out_pool = ctx.enter_context(tc.tile_pool(name="outp", bufs=4))
        acc_psum = ctx.enter_context(tc.tile_pool(name="accp", bufs=4, space="PSUM"))

        for ns in range(NSLAB):
            panel = panel_pool.tile([P, NMC * NW], bf16, tag="panel")
            # ScalarE HWDGE queue: prefetches ahead of the sync-queue traffic
            nc.scalar.dma_start(out=panel[:], in_=ah[ns * P : (ns + 1) * P, :])

            for db in range(NDB):
                xs = x_pool.tile([P, NMC * P], bf16, tag="xs")
                nc.sync.dma_start(out=xs[:], in_=xh[db * P : (db + 1) * P, :])

                acc = acc_psum.tile([P, NW], f32, tag="acc")
                for mc in range(NMC):
                    nc.tensor.matmul(
                        acc[:],
                        xs[:, mc * P : (mc + 1) * P],
                        panel[:, mc * NW : (mc + 1) * NW],
                        start=(mc == 0),
                        stop=(mc == NMC - 1),
                    )

                osb = out_pool.tile([P, NW], f32, tag="osb")
                nc.vector.tensor_copy(out=osb[:], in_=acc[:])
                nc.sync.dma_start(
                    out=out_ext[db * P : (db + 1) * P, ns * NW : (ns + 1) * NW],
                    in_=osb[:],
                )

    nc.compile()
    return nc


_NC_CACHE = {}


def _get_program():
    if "nc" not in _NC_CACHE:
        _NC_CACHE["nc"] = build_program()
    return _NC_CACHE["nc"]


def prepare_in_maps(x: np.ndarray, adj: np.ndarray) -> list:
    """Host-side prepack: transpose + tile + bf16-cast both operands."""
    import ml_dtypes

    bf16 = ml_dtypes.bfloat16

    xh_by_b = []
    for b in range(B):
        # [D, M] -> XT [M, D] bf16 -> [mc, k, db, i] -> [db, k, mc, i]
        xt = x[b].T.astype(bf16)  # [8192, 3072] contiguous copy
        xh = (
            xt.reshape(NMC, P, NDB, P)
            .transpose(2, 1, 0, 3)
            .reshape(D, NMC * P)
        )
        xh_by_b.append(np.ascontiguousarray(xh))

    in_maps = []
    for c in range(NCORES):
        b, q = divmod(c, NSPLIT)
        a = adj[b, q * NC : (q + 1) * NC, :].astype(bf16)  # [2048, 8192]
        # [ns, j, mc, k] -> [ns, k, mc, j]
        ah = (
            a.reshape(NSLAB, NW, NMC, P)
            .transpose(0, 3, 2, 1)
            .reshape(NSLAB * P, NMC * NW)
        )
        in_maps.append({"xh": xh_by_b[b], "ah": np.ascontiguousarray(ah)})
    return in_maps


def kernel(x: np.ndarray, adj: np.ndarray) -> np.ndarray:
    """Full inputs in, full output out. x [B,D,N] f32, adj [B,N,N] f32."""
    from concourse.bass_utils import run_bass_kernel_spmd

    assert x.shape == (B, D, N) and adj.shape == (B, N, N)
    nc = _get_program()
    in_maps = prepare_in_maps(np.asarray(x), np.asarray(adj))

    res = run_bass_kernel_spmd(nc, in_maps, core_ids=list(range(NCORES)))
    out = np.empty((B, D, N), dtype=np.float32)
    for c in range(NCORES):
        b, q = divmod(c, NSPLIT)
        out[b, :, q * NC : (q + 1) * NC] = res.results[c]["out"]
    return out


# revision 3
# speedup vs baseline: 1.6549x; 1.0045x over previous
"""Bass/Trainium2 kernel for batched GNN message passing:
    out[b, d, n] = sum_m adj[b, n, m] * x[b, d, m]
B=2, D=3072, N=8192, fp32 in/out.

Sharding: 8 cores, core c -> (b = c//4, n-quarter = c%4). Each core computes
C[3072, 2048] = X[b] @ A[b, quarter, :].T with contraction m = 8192.

Strategy (bf16, zero on-chip transposes, zero DRAM partials):
- Host prepacks both operands transposed + tiled so every DMA is contiguous
  and every matmul operand is already in [contraction-on-partitions] layout.
  bf16 rounding gives rel err ~2e-3 vs the 2e-2 gate (measured on the real
  seeded inputs).
- Per core: 4 n-slabs of 512 cols. Per slab, the full-contraction adj panel
  [128k x 64mc x 512n] (64 KiB/partition) is SBUF-resident (double-buffered
  across slabs -> no PE stall at slab swap). For each of 24 d-blocks, one
  PSUM bank accumulates out[128d, 512n] over all 64 mc chunks in a dense
  back-to-back matmul stream (LDWEIGHTS hides in the PE reorder window, HAM
  stays warm). X d-block strips re-stream per slab (4x50MB, hidden under
  compute).
- Evict: PSUM -> VectorE copy -> SBUF -> DMA out. Panel DMAs issue from the
  ScalarE HWDGE queue so they prefetch ahead of the x/out Sync-queue traffic.
"""

import sys
from contextlib import ExitStack

import numpy as np

sys.path.insert(0, "/opt/trn_rl_repo")

B = 2
D = 3072
N = 8192
NCORES = 8
NSPLIT = 4  # n-quarters per batch sample
NC = N // NSPLIT  # 2048 columns of out per core

P = 128
NDB = D // P  # 24 d-blocks
NMC = N // P  # 64 contraction chunks
NSLAB = 4  # n-slabs per core
NW = NC // NSLAB  # 512 cols per slab


def build_program():
    """Build the per-core Bass program. Returns compiled nc."""
    import concourse.mybir as mybir
    import concourse.tile as tile
    from concourse import bacc

    f32 = mybir.dt.float32
    bf16 = mybir.dt.bfloat16

    nc = bacc.Bacc(None, target_bir_lowering=False, debug=False)

    # xh[db*128 + k, mc*128 + i] = x[b][db*128 + i, mc*128 + k]  (bf16)
    xh = nc.dram_tensor("xh", [D, NMC * P], bf16, kind="ExternalInput")
    # ah[ns*128 + k, mc*512 + j] = adj[b][q*2048 + ns*512 + j, mc*128 + k]
    ah = nc.dram_tensor("ah", [NSLAB * P, NMC * NW], bf16, kind="ExternalInput")
    out_ext = nc.dram_tensor("out", [D, NC], f32, kind="ExternalOutput")

    with tile.TileContext(nc) as tc, ExitStack() as ctx:
        panel_pool = ctx.enter_context(tc.tile_pool(name="panel", bufs=2))
        x_pool = ctx.enter_context(tc.tile_pool(name="xp", bufs=3))
        out_pool = ctx.enter_context(tc.tile_pool(name="outp", bufs=4))
        acc_psum = ctx.enter_context(tc.tile_pool(name="accp", bufs=4, space="PSUM"))

        # DMAs are split into ~1MB pieces: Tile tracks sub-range deps, so the
        # first matmuls start as soon as the first chunk lands instead of
        # stalling ~30us on the full panel transfer.
        PANEL_PIECES = 8
        ppw = NMC * NW // PANEL_PIECES
        X_PIECES = 2
        xpw = NMC * P // X_PIECES

        for ns in range(NSLAB):
            panel = panel_pool.tile([P, NMC * NW], bf16, tag="panel")
            # ScalarE HWDGE queue: prefetches ahead of the sync-queue traffic
            for pp in range(PANEL_PIECES):
                nc.scalar.dma_start(
                    out=panel[:, pp * ppw : (pp + 1) * ppw],
                    in_=ah[ns * P : (ns + 1) * P, pp * ppw : (pp + 1) * ppw],
                )

            for db in range(NDB):
                xs = x_pool.tile([P, NMC * P], bf16, tag="xs")
                for xp in range(X_PIECES):
                    nc.sync.dma_start(
                        out=xs[:, xp * xpw : (xp + 1) * xpw],
                        in_=xh[db * P : (db + 1) * P, xp * xpw : (xp + 1) * xpw],
                    )

                acc = acc_psum.tile([P, NW], f32, tag="acc")
                for mc in range(NMC):
                    nc.tensor.matmul(
                        acc[:],
                        xs[:, mc * P : (mc + 1) * P],
                        panel[:, mc * NW : (mc + 1) * NW],
                        start=(mc == 0),
                        stop=(mc == NMC - 1),
                    )

                osb = out_pool.tile([P, NW], f32, tag="osb")
                nc.vector.tensor_copy(out=osb[:], in_=acc[:])
                nc.sync.dma_start(
                    out=out_ext[db * P : (db + 1) * P, ns * NW : (ns + 1) * NW],
                    in_=osb[:],
                )

    nc.compile()
    return nc


_NC_CACHE = {}


def _get_program():
    if "nc" not in _NC_CACHE:
        _NC_CACHE["nc"] = build_program()
    return _NC_CACHE["nc"]


def prepare_in_maps(x: np.ndarray, adj: np.ndarray) -> list:
    """Host-side prepack: transpose + tile + bf16-cast both operands."""
    import ml_dtypes

    bf16 = ml_dtypes.bfloat16

    xh_by_b = []
    for b in range(B):
        # [D, M] -> XT [M, D] bf16 -> [mc, k, db, i] -> [db, k, mc, i]
        xt = x[b].T.astype(bf16)  # [8192, 3072] contiguous copy
        xh = (
            xt.reshape(NMC, P, NDB, P)
            .transpose(2, 1, 0, 3)
            .reshape(D, NMC * P)
        )
        xh_by_b.append(np.ascontiguousarray(xh))

    in_maps = []
    for c in range(NCORES):
        b, q = divmod(c, NSPLIT)
        a = adj[b, q * NC : (q + 1) * NC, :].astype(bf16)  # [2048, 8192]
        # [ns, j, mc, k] -> [ns, k, mc, j]
        ah = (
            a.reshape(NSLAB, NW, NMC, P)
            .transpose(0, 3, 2, 1)
            .reshape(NSLAB * P, NMC * NW)
        )
        in_maps.append({"xh": xh_by_b[b], "ah": np.ascontiguousarray(ah)})
    return in_maps


def kernel(x: np.ndarray, adj: np.ndarray) -> np.ndarray:
    """Full inputs in, full output out. x [B,D,N] f32, adj [B,N,N] f32."""
    from concourse.bass_utils import run_bass_kernel_spmd

    assert x.shape == (B, D, N) and adj.shape == (B, N, N)
    nc = _get_program()
    in_maps = prepare_in_maps(np.asarray(x), np.asarray(adj))

    res = run_bass_kernel_spmd(nc, in_maps, core_ids=list(range(NCORES)))
    out = np.empty((B, D, N), dtype=np.float32)
    for c in range(NCORES):
        b, q = divmod(c, NSPLIT)
        out[b, :, q * NC : (q + 1) * NC] = res.results[c]["out"]
    return out


# revision 4
# speedup vs baseline: 1.6663x; 1.0069x over previous
"""Bass/Trainium2 kernel for batched GNN message passing:
    out[b, d, n] = sum_m adj[b, n, m] * x[b, d, m]
B=2, D=3072, N=8192, fp32 in/out.

Sharding: 8 cores, core c -> (b = c//4, n-quarter = c%4). Each core computes
C[3072, 2048] = X[b] @ A[b, quarter, :].T with contraction m = 8192.

Strategy (bf16, zero on-chip transposes, zero DRAM partials):
- Host prepacks both operands transposed + tiled so every DMA is contiguous
  and every matmul operand is already in [contraction-on-partitions] layout.
  bf16 rounding gives rel err ~2e-3 vs the 2e-2 gate (measured on the real
  seeded inputs).
- Per core: 4 n-slabs of 512 cols. Per slab, the full-contraction adj panel
  [128k x 64mc x 512n] (64 KiB/partition) is SBUF-resident (double-buffered
  across slabs -> no PE stall at slab swap). For each of 24 d-blocks, one
  PSUM bank accumulates out[128d, 512n] over all 64 mc chunks in a dense
  back-to-back matmul stream (LDWEIGHTS hides in the PE reorder window, HAM
  stays warm). X d-block strips re-stream per slab (4x50MB, hidden under
  compute).
- Evict: PSUM -> VectorE copy -> SBUF -> DMA out. Panel DMAs issue from the
  ScalarE HWDGE queue so they prefetch ahead of the x/out Sync-queue traffic.
"""

import sys
from contextlib import ExitStack

import numpy as np

sys.path.insert(0, "/opt/trn_rl_repo")

B = 2
D = 3072
N = 8192
NCORES = 8
NSPLIT = 4  # n-quarters per batch sample
NC = N // NSPLIT  # 2048 columns of out per core

P = 128
NDB = D // P  # 24 d-blocks
NMC = N // P  # 64 contraction chunks
NSLAB = 4  # n-slabs per core
NW = NC // NSLAB  # 512 cols per slab


def build_program():
    """Build the per-core Bass program. Returns compiled nc."""
    import concourse.mybir as mybir
    import concourse.tile as tile
    from concourse import bacc

    f32 = mybir.dt.float32
    bf16 = mybir.dt.bfloat16

    nc = bacc.Bacc(None, target_bir_lowering=False, debug=False)

    # xh[db*128 + k, mc*128 + i] = x[b][db*128 + i, mc*128 + k]  (bf16)
    xh = nc.dram_tensor("xh", [D, NMC * P], bf16, kind="ExternalInput")
    # ah[ns*128 + k, mc*512 + j] = adj[b][q*2048 + ns*512 + j, mc*128 + k]
    ah = nc.dram_tensor("ah", [NSLAB * P, NMC * NW], bf16, kind="ExternalInput")
    out_ext = nc.dram_tensor("out", [D, NC], f32, kind="ExternalOutput")

    with tile.TileContext(nc) as tc, ExitStack() as ctx:
        panel_pool = ctx.enter_context(tc.tile_pool(name="panel", bufs=2))
        x_pool = ctx.enter_context(tc.tile_pool(name="xp", bufs=3))
        out_pool = ctx.enter_context(tc.tile_pool(name="outp", bufs=4))
        acc_psum = ctx.enter_context(tc.tile_pool(name="accp", bufs=4, space="PSUM"))

        # DMAs are split into pieces: Tile tracks sub-range deps, so matmuls
        # start as soon as the first chunk lands instead of stalling ~30us on
        # the full panel transfer. Tensor-engine instructions execute in
        # program order, so emission order is the PE schedule.

        def load_x(db, pieces):
            xs = x_pool.tile([P, NMC * P], bf16, tag="xs")
            xpw = NMC * P // pieces
            for xp in range(pieces):
                nc.sync.dma_start(
                    out=xs[:, xp * xpw : (xp + 1) * xpw],
                    in_=xh[db * P : (db + 1) * P, xp * xpw : (xp + 1) * xpw],
                )
            return xs

        def mm_group(acc, xs, panel, mcs):
            for mc in mcs:
                nc.tensor.matmul(
                    acc[:],
                    xs[:, mc * P : (mc + 1) * P],
                    panel[:, mc * NW : (mc + 1) * NW],
                    start=(mc == 0),
                    stop=(mc == NMC - 1),
                )

        def evict(acc, db, ns):
            osb = out_pool.tile([P, NW], f32, tag="osb")
            nc.vector.tensor_copy(out=osb[:], in_=acc[:])
            nc.sync.dma_start(
                out=out_ext[db * P : (db + 1) * P, ns * NW : (ns + 1) * NW],
                in_=osb[:],
            )

        for ns in range(NSLAB):
            panel = panel_pool.tile([P, NMC * NW], bf16, tag="panel")
            # ScalarE HWDGE queue: prefetches ahead of the sync-queue traffic
            pieces = 16 if ns == 0 else 8
            ppw = NMC * NW // pieces
            for pp in range(pieces):
                nc.scalar.dma_start(
                    out=panel[:, pp * ppw : (pp + 1) * ppw],
                    in_=ah[ns * P : (ns + 1) * P, pp * ppw : (pp + 1) * ppw],
                )

            if ns == 0:
                # Startup: panel-0 streams in at HBM pace (~25us), slower than
                # one d-block's matmuls (13.7us). Interleave the first two
                # d-blocks piece-by-piece so the PE stays busy throughout.
                xs0 = load_x(0, 4)
                xs1 = load_x(1, 4)
                acc0 = acc_psum.tile([P, NW], f32, tag="acc")
                acc1 = acc_psum.tile([P, NW], f32, tag="acc")
                mc_per_piece = NMC // 16
                for pp in range(16):
                    mcs = range(pp * mc_per_piece, (pp + 1) * mc_per_piece)
                    mm_group(acc0, xs0, panel, mcs)
                    mm_group(acc1, xs1, panel, mcs)
                evict(acc0, 0, ns)
                evict(acc1, 1, ns)
                rest = range(2, NDB)
            else:
                rest = range(NDB)

            for db in rest:
                xs = load_x(db, 2)
                acc = acc_psum.tile([P, NW], f32, tag="acc")
                mm_group(acc, xs, panel, range(NMC))
                evict(acc, db, ns)

    nc.compile()
    return nc


_NC_CACHE = {}


def _get_program():
    if "nc" not in _NC_CACHE:
        _NC_CACHE["nc"] = build_program()
    return _NC_CACHE["nc"]


def prepare_in_maps(x: np.ndarray, adj: np.ndarray) -> list:
    """Host-side prepack: transpose + tile + bf16-cast both operands."""
    import ml_dtypes

    bf16 = ml_dtypes.bfloat16

    xh_by_b = []
    for b in range(B):
        # [D, M] -> XT [M, D] bf16 -> [mc, k, db, i] -> [db, k, mc, i]
        xt = x[b].T.astype(bf16)  # [8192, 3072] contiguous copy
        xh = (
            xt.reshape(NMC, P, NDB, P)
            .transpose(2, 1, 0, 3)
            .reshape(D, NMC * P)
        )
        xh_by_b.append(np.ascontiguousarray(xh))

    in_maps = []
    for c in range(NCORES):
        b, q = divmod(c, NSPLIT)
        a = adj[b, q * NC : (q + 1) * NC, :].astype(bf16)  # [2048, 8192]
        # [ns, j, mc, k] -> [ns, k, mc, j]
        ah = (
            a.reshape(NSLAB, NW, NMC, P)
            .transpose(0, 3, 2, 1)
            .reshape(NSLAB * P, NMC * NW)
        )
        in_maps.append({"xh": xh_by_b[b], "ah": np.ascontiguousarray(ah)})
    return in_maps


def kernel(x: np.ndarray, adj: np.ndarray) -> np.ndarray:
    """Full inputs in, full output out. x [B,D,N] f32, adj [B,N,N] f32."""
    from concourse.bass_utils import run_bass_kernel_spmd

    assert x.shape == (B, D, N) and adj.shape == (B, N, N)
    nc = _get_program()
    in_maps = prepare_in_maps(np.asarray(x), np.asarray(adj))

    res = run_bass_kernel_spmd(nc, in_maps, core_ids=list(range(NCORES)))
    out = np.empty((B, D, N), dtype=np.float32)
    for c in range(NCORES):
        b, q = divmod(c, NSPLIT)
        out[b, :, q * NC : (q + 1) * NC] = res.results[c]["out"]
    return out
